# revision 5
# baseline (speedup 1.0000x reference)
"""Trainium2 Bass kernel for BatchMultiHeadGraphAttention (OAG-style GAT).

Reference computation (per batch b, head k):
    hp   = h @ w[k]                               # [n, 64]
    t    = tanh(hp)
    src  = sum_o t[:, o] * a_src[k][o, type(n)]   # [n]
    dst  = sum_o t[:, o] * a_dst[k][o, type(n)]   # [n]
    attn = softmax_j( mask(adj, leaky_relu(src_i + dst_j, 0.2)) )
    out  = attn @ hp + bias

Key identities used on-chip (x = src_i + dst_j):
    exp(lrelu(x)) = max(exp(x), exp(0.2 x))
and softmax is invariant to any per-row (per-i) positive scaling, so dividing
by exp(src_i) gives the streamed matrix
    A[j, i] = adjT[j, i] * max( F1[j],  W[i] * F2[j] )
with F1 = exp(dst), F2 = exp(0.2 dst) per-partition scalars and
W = exp(-0.8 src) broadcast along partitions.  That is ONE dual-op
tensor_scalar (4x mode) + ONE masking tensor_tensor per 128x2048 tile.

The value matmul keeps [hp | ones ones] stationary and streams A, producing
OUT.T[o, i] in PSUM with softmax denominators in the ones-rows; a PE
transpose + per-partition reciprocal scale (on ScalarE) finishes the head.
Because attention rows sum to 1 the bias is a plain additive term and is
applied on the host after gathering.

Sharding: core c <- batch b = c // 2, heads (c % 2) * 4 ... + 4.  The
adjacency matrix is transposed and cast to bf16 on the host so it streams
into SBUF in the [key-partition, query-free] layout the kernel needs.
"""

import numpy as np
import ml_dtypes

import concourse.bass as bass
import concourse.mybir as mybir
import concourse.tile as tile
from concourse import bacc
from concourse.bass_utils import run_bass_kernel_spmd
from concourse.masks import make_identity

F32 = mybir.dt.float32
BF16 = mybir.dt.bfloat16
AF = mybir.ActivationFunctionType
OP = mybir.AluOpType

N = 2048          # nodes
F_IN = 128        # input features
F_OUT = 64        # output features
NTYPE = 3         # node types
KH = 4            # heads per core
NT = N // 128     # 16 node tiles
M1 = F_OUT + 2    # stationary width: 64 hp cols + 2 ones cols

N_CORES = 8
BS = 4
N_HEAD = 8

# Mask multiplies are split DVE/GPSIMD (HW-measured GPSIMD tensor_tensor
# bf16 is ~1 us per 128x2048 tile -- far cheaper than the cost model
# claims).  DVE also runs all the gens (tensor_scalar 4x mode), so it only
# takes DVE_MASKS of the 16 mask tiles per head; GPSIMD takes the rest.
DVE_MASKS = 5
DVE_JTS = frozenset(round((k + 0.5) * 16 / DVE_MASKS - 0.5) for k in range(DVE_MASKS))


def build_bass(finalize=True, repeat=1):
    nc = bacc.Bacc("TRN2", target_bir_lowering=False)

    h_d = nc.dram_tensor("h", [N, F_IN], F32, kind="ExternalInput")
    adjT_d = nc.dram_tensor("adjT", [N, N], BF16, kind="ExternalInput")
    vtT_d = nc.dram_tensor("vtT", [NTYPE, N], F32, kind="ExternalInput")
    w_d = nc.dram_tensor("w", [KH, F_IN, F_OUT], F32, kind="ExternalInput")
    asT_d = nc.dram_tensor("a_srcT", [KH, NTYPE, F_OUT], F32, kind="ExternalInput")
    adT_d = nc.dram_tensor("a_dstT", [KH, NTYPE, F_OUT], F32, kind="ExternalInput")
    out_d = nc.dram_tensor("out", [KH, N, F_OUT], F32, kind="ExternalOutput")

    with tile.TileContext(nc) as tc:
        with (
            tc.tile_pool(name="const", bufs=1) as cpool,
            tc.tile_pool(name="ph", bufs=2) as ph,
            tc.tile_pool(name="ph4", bufs=4) as ph4,
            tc.tile_pool(name="ph1", bufs=1) as ph1,
            tc.tile_pool(name="amain", bufs=3) as ap_,
            tc.tile_pool(name="ammask", bufs=5) as amp,
            tc.tile_pool(name="psA", bufs=1, space="PSUM") as psA,
            tc.tile_pool(name="psOut", bufs=1, space="PSUM") as psOut,
        ):
            # ---------------- constants / inputs ----------------
            ident = cpool.tile([128, 128], F32, tag="ident")
            make_identity(nc, ident)
            ident_bf = cpool.tile([128, 128], BF16, tag="ident_bf")
            nc.vector.tensor_copy(ident_bf, ident)

            # 0/1 block matrices: OnesH[h].T @ smul2 sums a head's 64
            # o-partitions AND broadcasts the result across all 128 output
            # partitions in a single matmul (reduce+broadcast fused)
            ones_h = []
            for h in range(2):
                t_ = cpool.tile([128, 128], BF16, tag=f"ones_h{h}")
                nc.vector.memset(t_, 0.0)
                nc.vector.memset(t_[h * F_OUT : (h + 1) * F_OUT, :], 1.0)
                ones_h.append(t_)

            # latency-critical inputs first, bulk adjacency last
            h_sb = ph1.tile([128, NT, F_IN], F32, tag="tanhT2")
            h_re = h_d.ap().rearrange("(t p) f -> p t f", p=128)
            for g in range(4):
                nc.sync.dma_start(
                    out=h_sb[:, 4 * g : 4 * (g + 1), :],
                    in_=h_re[:, 4 * g : 4 * (g + 1), :],
                )
            vtT_sb = cpool.tile([NTYPE, N], F32, tag="vtT")
            nc.sync.dma_start(out=vtT_sb, in_=vtT_d.ap())
            adT_sb = cpool.tile([NTYPE, KH, F_OUT], F32, tag="adT")
            nc.sync.dma_start(out=adT_sb, in_=adT_d.ap().rearrange("k t o -> t k o"))
            asT_sb = cpool.tile([NTYPE, KH, F_OUT], F32, tag="asT")
            nc.sync.dma_start(out=asT_sb, in_=asT_d.ap().rearrange("k t o -> t k o"))
            w_sb = cpool.tile([128, KH, F_OUT], F32, tag="w_sb")
            nc.sync.dma_start(out=w_sb, in_=w_d.ap().rearrange("k f o -> f k o"))

            adjT_sb = cpool.tile([128, NT, N], BF16, tag="adjT")

            hT = cpool.tile([128, N], F32, tag="hT")

            def emit_selects(pair):
                """Type-select matrices for both heads of a pair; these only
                need the small inputs, so they can fill the PE early."""
                k0 = 2 * pair
                ps_aselN2 = psA.tile([128, NT, 128], F32, tag="psA")
                for t in range(NT):
                    nc.tensor.matmul(
                        ps_aselN2[:, t, :],
                        lhsT=vtT_sb[:, t * 128 : (t + 1) * 128],
                        rhs=adT_sb[:, k0 : k0 + 2, :],
                        start=True, stop=True,
                    )
                aselN2 = ph1.tile([128, NT, 128], BF16, tag="aselN2")
                nc.scalar.copy(aselN2, ps_aselN2)

                ps_asel2 = psA.tile([128, N], F32, tag="psA")
                for i in range(4):
                    sl = slice(i * 512, (i + 1) * 512)
                    nc.tensor.matmul(
                        ps_asel2[:, sl], lhsT=asT_sb[:, k0 : k0 + 2, :],
                        rhs=vtT_sb[:, sl], start=True, stop=True,
                    )
                asel2 = ph1.tile([128, N], BF16, tag="asel2")
                nc.scalar.copy(asel2, ps_asel2)
                return aselN2, asel2

            # pair 0 selects before the hT transposes: PE works while the
            # h DMA is in flight
            selects0 = emit_selects(0)

            # bulk adjacency load: issued from the otherwise-idle sync
            # queue AFTER the startup-critical loads (DMA issue from
            # scalar/gpsimd would steal those engines' sequencer time);
            # first needed by the jt=0 mask ~50 us in
            for t in range(NT):
                nc.sync.dma_start(
                    out=adjT_sb[:, t, :], in_=adjT_d[t * 128 : (t + 1) * 128, :]
                )

            # hT[f, n] = h.T via PE transposes
            ps_hT = psA.tile([128, N], F32, tag="psA")
            for t in range(NT):
                nc.tensor.transpose(
                    ps_hT[:, t * 128 : (t + 1) * 128], h_sb[:, t, :], ident
                )
            for i in range(4):
                sl = slice(i * 512, (i + 1) * 512)
                nc.scalar.copy(hT[:, sl], ps_hT[:, sl])

            def setup_pair_a(pair, selects=None):
                """PE/ACT-only prologue of a pair (no DVE instructions, so it
                can be emitted ahead without blocking the DVE stream)."""
                k0 = 2 * pair
                aselN2, asel2 = selects if selects else emit_selects(pair)

                # hpT2[2*64, n]: heads k0, k0+1 stacked on partitions
                ps_hpT2 = psA.tile([128, N], F32, tag="psA")
                for i in range(4):
                    sl = slice(i * 512, (i + 1) * 512)
                    nc.tensor.matmul(
                        ps_hpT2[:, sl], lhsT=w_sb[:, k0 : k0 + 2, :],
                        rhs=hT[:, sl], start=True, stop=True,
                    )
                tanhT2 = ph1.tile([128, N], BF16, tag="tanhT2")
                hpT2sb = ph1.tile([128, N], BF16, tag="hpT2sb")
                for i in range(4):
                    sl = slice(i * 512, (i + 1) * 512)
                    nc.scalar.activation(tanhT2[:, sl], ps_hpT2[:, sl], AF.Tanh)
                    nc.scalar.copy(hpT2sb[:, sl], ps_hpT2[:, sl])

                # hp2[n, 2*64] via PE transposes of hpT2
                ps_hp2 = psA.tile([128, NT, 128], BF16, tag="psA")
                for t in range(NT):
                    nc.tensor.transpose(
                        ps_hp2[:, t, :], hpT2sb[:, t * 128 : (t + 1) * 128],
                        ident_bf,
                    )
                tanh_hp2 = ph1.tile([128, NT, 128], BF16, tag="tanh_hp2")
                nc.scalar.activation(tanh_hp2, ps_hp2, AF.Tanh)
                hp1 = []
                for h in range(2):
                    t_ = ph4.tile([128, NT, M1], BF16, tag="hp1")
                    nc.gpsimd.memset(t_[:, :, F_OUT:M1], 1.0)
                    nc.scalar.copy(
                        t_[:, :, 0:F_OUT],
                        ps_hp2[:, :, h * F_OUT : (h + 1) * F_OUT],
                    )
                    hp1.append(t_)
                return dict(
                    k0=k0, aselN2=aselN2, asel2=asel2, tanhT2=tanhT2,
                    tanh_hp2=tanh_hp2, hp1=hp1,
                )

            def setup_pair_b(actx):
                """DVE-dependent tail of the pair setup."""
                smul2 = ph1.tile([128, N], BF16, tag="smul2")
                for i in range(4):
                    sl = slice(i * 512, (i + 1) * 512)
                    nc.vector.tensor_tensor(
                        smul2[:, sl], actx["tanhT2"][:, sl],
                        actx["asel2"][:, sl], op=OP.mult,
                    )

                # W[i] = exp(-0.8 src_i) broadcast across partitions;
                # src-sum and broadcast come out of one matmul per head
                Wb = []
                for h in range(2):
                    ps_sraw = psA.tile([128, N], F32, tag="psA")
                    for i in range(4):
                        sl = slice(i * 512, (i + 1) * 512)
                        nc.tensor.matmul(
                            ps_sraw[:, sl], lhsT=ones_h[h], rhs=smul2[:, sl],
                            start=True, stop=True,
                        )
                    wb = ph.tile([128, N], BF16, tag=f"Wb{h}")
                    nc.scalar.activation(wb, ps_sraw, AF.Exp, scale=-0.8)
                    Wb.append(wb)

                # dst scalars
                dmul2 = ph1.tile([128, NT, 128], BF16, tag="dmul2")
                nc.vector.tensor_tensor(
                    dmul2, actx["tanh_hp2"], actx["aselN2"], op=OP.mult
                )
                dstc2 = ph.tile([128, NT, 2], F32, tag="dstc2")
                nc.vector.tensor_reduce(
                    dstc2, dmul2.rearrange("p t (h o) -> p t h o", h=2),
                    axis=mybir.AxisListType.X, op=OP.add,
                )
                F1_2 = ph.tile([128, NT, 2], F32, tag="F1_2")
                nc.scalar.activation(F1_2, dstc2, AF.Exp)
                F2_2 = ph.tile([128, NT, 2], F32, tag="F2_2")
                nc.scalar.activation(F2_2, dstc2, AF.Exp, scale=0.2)
                return dict(Wb=Wb, hp1=actx["hp1"], F1_2=F1_2, F2_2=F2_2)

            def run_head(ctxh, k0, h):
                Wb, hp1 = ctxh["Wb"], ctxh["hp1"]
                F1_2, F2_2 = ctxh["F1_2"], ctxh["F2_2"]

                ps_outT = psOut.tile([M1, N], F32, tag="outT")
                for jt in range(NT):
                    A = ap_.tile([128, N], BF16, tag="A")
                    # A = max(W * F2[j], F1[j]) -- one 4x-mode op
                    nc.vector.tensor_scalar(
                        A, Wb[h],
                        F2_2[:, jt, h : h + 1], F1_2[:, jt, h : h + 1],
                        op0=OP.mult, op1=OP.max,
                    )
                    Am = amp.tile([128, N], BF16, tag="Am")
                    eng = nc.vector if jt in DVE_JTS else nc.gpsimd
                    eng.tensor_tensor(Am, A, adjT_sb[:, jt, :], op=OP.mult)
                    for i in range(4):
                        sl = slice(i * 512, (i + 1) * 512)
                        nc.tensor.matmul(
                            ps_outT[:, sl], lhsT=hp1[h][:, jt, :],
                            rhs=Am[:, sl],
                            start=(jt == 0), stop=(jt == NT - 1),
                        )

                # free the accumulator fast; the rest of the epilogue is
                # emitted later (after the next head's main loop) so its
                # transpose-wait doesn't stall the DVE stream
                outT_sb = ph.tile([M1, N], F32, tag="outT_sb")
                nc.scalar.copy(outT_sb, ps_outT)

                def finish():
                    ps_tr = psA.tile([128, NT, 128], F32, tag="psA")
                    for ic in range(NT):
                        nc.tensor.transpose(
                            ps_tr[:, ic, 0:M1],
                            outT_sb[:, ic * 128 : (ic + 1) * 128],
                            ident[0:M1, 0:M1],
                        )
                    recip = ph.tile([128, NT], F32, tag="recip")
                    nc.vector.reciprocal(recip, ps_tr[:, :, F_OUT])
                    outf = ph.tile([128, NT, F_OUT], F32, tag="outf")
                    for ic in range(NT):
                        nc.scalar.activation(
                            outf[:, ic, :], ps_tr[:, ic, 0:F_OUT], AF.Copy,
                            scale=recip[:, ic : ic + 1],
                        )
                    nc.sync.dma_start(
                        out=out_d[k0 + h].rearrange("(t p) o -> p t o", p=128),
                        in_=outf,
                    )
                return finish

            # emission order: pair-1's PE/ACT prologue goes ahead of the
            # pair-0 main loops (fills PE/ACT idle time without inserting
            # anything into the DVE stream); its DVE tail lands between the
            # two pair-0 heads.  `repeat` re-runs the whole computation for
            # slope-based timing (dispatch overhead cancels).
            for rep in range(repeat):
                a0 = setup_pair_a(0, selects0 if rep == 0 else None)
                ctx0 = setup_pair_b(a0)
                a1 = setup_pair_a(1)
                f00 = run_head(ctx0, 0, 0)
                ctx1 = setup_pair_b(a1)
                f01 = run_head(ctx0, 0, 1)
                f00()
                f10 = run_head(ctx1, 2, 0)
                f01()
                f11 = run_head(ctx1, 2, 1)
                f10()
                f11()

    if finalize:
        nc.finalize()
    return nc


_NC = None


def _get_nc():
    global _NC
    if _NC is None:
        _NC = build_bass()
    return _NC


def build_in_maps(np_inputs):
    h = np.asarray(np_inputs["h"], dtype=np.float32)
    adj = np.asarray(np_inputs["adj"])
    v_types = np.asarray(np_inputs["v_types"], dtype=np.float32)
    w = np.asarray(np_inputs["w"], dtype=np.float32)
    a_src = np.asarray(np_inputs["a_src"], dtype=np.float32)
    a_dst = np.asarray(np_inputs["a_dst"], dtype=np.float32)

    in_maps = []
    for c in range(N_CORES):
        b = c // 2
        k0 = (c % 2) * KH
        in_maps.append({
            "h": np.ascontiguousarray(h[b]),
            "adjT": np.ascontiguousarray(adj[b].T).astype(ml_dtypes.bfloat16),
            "vtT": np.ascontiguousarray(v_types[b].T),
            "w": np.ascontiguousarray(w[k0 : k0 + KH]),
            "a_srcT": np.ascontiguousarray(a_src[k0 : k0 + KH].transpose(0, 2, 1)),
            "a_dstT": np.ascontiguousarray(a_dst[k0 : k0 + KH].transpose(0, 2, 1)),
        })
    return in_maps


last_results = None  # BassKernelResults of the most recent kernel() call


def kernel(h, adj, v_types, w, a_src, a_dst, bias, _trace=False):
    nc = _get_nc()
    in_maps = build_in_maps(dict(
        h=h, adj=adj, v_types=v_types, w=w, a_src=a_src, a_dst=a_dst
    ))

    res = run_bass_kernel_spmd(
        nc, in_maps, core_ids=list(range(N_CORES)), trace=_trace
    )
    global last_results
    last_results = res

    out = np.empty((BS, N_HEAD, N, F_OUT), dtype=np.float32)
    for c in range(N_CORES):
        b = c // 2
        k0 = (c % 2) * KH
        out[b, k0 : k0 + KH] = res.results[c]["out"]
    # attention rows sum to 1, so the bias is a plain additive term; adding it
    # on the host keeps the device epilogue a pure copy-scale
    bias = np.asarray(bias, dtype=np.float32)
    if bias.any():
        out += bias
    return out

